# revision 1
# baseline (speedup 1.0000x reference)
"""nn_Attention TRN2 kernel: multi-head attention, tensor-parallel over heads
across 8 NeuronCores.

Contract: kernel(**inputs) takes the FULL unsharded inputs
  x [2, 2048, 1024] f32, w_qkv [1024, 3072] f32, w_out [1024, 1024] f32,
  b_out [1024] f32
and returns the FULL output [2, 2048, 1024] f32.

Sharding: 16 heads / 8 cores = 2 heads per core (tensor parallel). Each core
computes qkv projections for its 2 heads, attention, and its partial
contribution to the output projection; the host sums the 8 partials + bias.

Per-core schedule (v2): engines balanced around PE ~135us and Act(exp)
~133us floors —
  - host supplies xT (x pre-transposed, bf16); all matmul operands bf16
  - b0: K projections first (scores need all of K), then Q(ic0), then V;
    attention(b0) starts as early as possible
  - b1 K/V projections interleaved into attention(b0)'s window; Q chunks
    just-in-time; b0 out-projections deferred into attention(b1)'s window
    (fills PE during Activation-bound softmax stretches)
  - scores per j-tile: the two heads' K=64 matmuls auto-place at PE
    row-tiles (0,0)/(64,0) and run concurrently
  - softmax: no max-subtraction (scores ~ N(0,1)); exp on ScalarE
    PSUM->SBUF bf16; denominators via a ones-column appended to V so the
    PV matmul emits them for free
  - normalize: DVE reciprocal of the denominator row -> GPSIMD
    partition_broadcast -> DVE multiply (no PE broadcast matmul)
  - out projection: attn [128,128] bf16 stationary x wo [128,512] bf16
    moving; partial [4096, 1024] bf16 DMA'd out per core
"""
from contextlib import ExitStack

import numpy as np

_CACHE = {}

B = 2
S = 2048
D = 1024
M = B * S
DH = 64
HC = 2
NH = HC * DH
KI = 128
KO = D // KI
MC = 512
NMC = S // MC
IC = 512
NIC = S // IC
NJT = S // KI
EXPW = 2 * IC
N_CORES = 8


def _build_kernel():
    import concourse.tile as tile
    from concourse import bacc, mybir
    from concourse.masks import make_identity

    F32 = mybir.dt.float32
    BF16 = mybir.dt.bfloat16

    nc = bacc.Bacc("TRN2", target_bir_lowering=False, debug=False,
                   num_devices=N_CORES)
    xT = nc.dram_tensor("xT", [D, M], BF16, kind="ExternalInput").ap()
    wq = nc.dram_tensor("wq", [D, NH], BF16, kind="ExternalInput").ap()
    wk = nc.dram_tensor("wk", [D, NH], BF16, kind="ExternalInput").ap()
    wv = nc.dram_tensor("wv", [D, NH], BF16, kind="ExternalInput").ap()
    wo = nc.dram_tensor("wo", [NH, D], BF16, kind="ExternalInput").ap()
    part = nc.dram_tensor("part", [M, D], BF16, kind="ExternalOutput").ap()

    with tile.TileContext(nc, trace_sim=False) as tc:
        with ExitStack() as ctx:
            persist = ctx.enter_context(tc.tile_pool(name="persist", bufs=1))
            xtp = ctx.enter_context(tc.tile_pool(name="xtp", bufs=4))
            vtmp = ctx.enter_context(tc.tile_pool(name="vtmp", bufs=2))
            expp = ctx.enter_context(tc.tile_pool(name="expp", bufs=2))
            attnp = ctx.enter_context(tc.tile_pool(name="attnp", bufs=6))
            rcpp = ctx.enter_context(tc.tile_pool(name="rcpp", bufs=2))
            bcp = ctx.enter_context(tc.tile_pool(name="bcp", bufs=2))
            osb = ctx.enter_context(tc.tile_pool(name="osb", bufs=3))
            # PSUM budget (8 banks): psA 2x1 + sps 2x2 + mix 2x1
            psA = ctx.enter_context(
                tc.tile_pool(name="psA", bufs=2, space="PSUM"))
            sps = ctx.enter_context(
                tc.tile_pool(name="sps", bufs=2, space="PSUM"))
            mix = ctx.enter_context(
                tc.tile_pool(name="mix", bufs=2, space="PSUM"))

            wq_sb = persist.tile([KI, KO, NH], BF16)
            nc.sync.dma_start(
                wq_sb[:], wq.rearrange("(ko ki) n -> ki ko n", ki=KI))
            wk_sb = persist.tile([KI, KO, NH], BF16)
            nc.sync.dma_start(
                wk_sb[:], wk.rearrange("(ko ki) n -> ki ko n", ki=KI))
            wv_sb = persist.tile([KI, KO, NH], BF16)
            nc.sync.dma_start(
                wv_sb[:], wv.rearrange("(ko ki) n -> ki ko n", ki=KI))
            wo_sb = persist.tile([KI, D], BF16)
            nc.sync.dma_start(wo_sb[:], wo)

            identbf = persist.tile([KI, KI], BF16)
            make_identity(nc, identbf)

            qT_b, kT_b, v_b = [], [], []
            for b in range(B):
                qT_b.append(persist.tile([NH, NMC, MC], BF16,
                                         tag=f"qT{b}", name=f"qT{b}"))
                kT_b.append(persist.tile([NH, NMC, MC], BF16,
                                         tag=f"kT{b}", name=f"kT{b}"))
                vt = persist.tile([KI, NJT, 2 * (DH + 1)], BF16, tag=f"v{b}")
                nc.vector.memset(vt[:, :, DH], 1.0)
                nc.vector.memset(vt[:, :, 2 * DH + 1], 1.0)
                v_b.append(vt)

            def load_x(b, mc):
                xT_sb = xtp.tile([KI, KO, MC], BF16, tag="xT_sb")
                m0 = b * S + mc * MC
                nc.sync.dma_start(
                    xT_sb[:],
                    xT[:, m0:m0 + MC].rearrange("(ko ki) m -> ki ko m", ki=KI))
                return xT_sb

            def proj(w_sb, xT_sb, dst, mc):
                ps = psA.tile([NH, MC], F32, tag="psA")
                for ko in range(KO):
                    nc.tensor.matmul(ps[:], w_sb[:, ko], xT_sb[:, ko],
                                     start=(ko == 0), stop=(ko == KO - 1))
                nc.vector.tensor_copy(out=dst[:, mc], in_=ps[:])

            def v_proj(xT_sb, b, mc):
                ps = psA.tile([NH, MC], F32, tag="psA")
                for ko in range(KO):
                    nc.tensor.matmul(ps[:], wv_sb[:, ko], xT_sb[:, ko],
                                     start=(ko == 0), stop=(ko == KO - 1))
                vt_sb = vtmp.tile([NH, MC], BF16, tag="vt_sb")
                nc.vector.tensor_copy(out=vt_sb[:], in_=ps[:])
                tpv = psA.tile([KI, MC // KI, KI], BF16, tag="psA")
                for mt in range(MC // KI):
                    nc.tensor.matmul(
                        tpv[:, mt], vt_sb[:, mt * KI:(mt + 1) * KI],
                        identbf, is_transpose=True,
                        start=(mt == 0), stop=(mt == MC // KI - 1))
                for h in range(HC):
                    nc.vector.tensor_copy(
                        out=v_b[b][:, mc * (MC // KI):(mc + 1) * (MC // KI),
                                   h * (DH + 1):h * (DH + 1) + DH],
                        in_=tpv[:, :, h * DH:(h + 1) * DH])

            def attn_core(b, ic):
                exp_sb = expp.tile([KI, NJT, EXPW], BF16, tag="exp_sb")
                for jt in range(NJT):
                    s_ps = sps.tile([KI, EXPW], F32, tag="s_ps")
                    mcj, oj = jt // 4, (jt % 4) * KI
                    for h in range(HC):
                        nc.tensor.matmul(
                            s_ps[:, h * IC:(h + 1) * IC],
                            kT_b[b][h * DH:(h + 1) * DH, mcj, oj:oj + KI],
                            qT_b[b][h * DH:(h + 1) * DH, ic],
                            start=True, stop=True)
                    nc.scalar.activation(
                        exp_sb[:, jt], s_ps[:],
                        mybir.ActivationFunctionType.Exp)

                attn_sb = attnp.tile([NH, IC], BF16, tag="attn_sb")
                for h in range(HC):
                    pv = mix.tile([DH + 1, IC], F32, tag="mix")
                    for jt in range(NJT):
                        nc.tensor.matmul(
                            pv[:],
                            v_b[b][:, jt, h * (DH + 1):(h + 1) * (DH + 1)],
                            exp_sb[:, jt, h * IC:(h + 1) * IC],
                            start=(jt == 0), stop=(jt == NJT - 1))
                    recip32 = rcpp.tile([1, IC], F32, tag="recip32")
                    nc.vector.reciprocal(recip32[:], pv[DH:DH + 1, :])
                    bc_sb = bcp.tile([DH, IC], F32, tag="bc_sb")
                    nc.gpsimd.partition_broadcast(
                        bc_sb[:], recip32[:], channels=DH)
                    nc.vector.tensor_mul(
                        out=attn_sb[h * DH:(h + 1) * DH, :],
                        in0=pv[0:DH, :], in1=bc_sb[:])
                return attn_sb

            def out_proj(b, ic, attn_sb):
                for it in range(IC // KI):
                    for nk in range(D // 512):
                        op = psA.tile([KI, 512], F32, tag="psA")
                        nc.tensor.matmul(
                            op[:], attn_sb[:, it * KI:(it + 1) * KI],
                            wo_sb[:, nk * 512:(nk + 1) * 512],
                            start=True, stop=True)
                        o_sb = osb.tile([KI, 512], BF16, tag="o_sb")
                        nc.vector.tensor_copy(out=o_sb[:], in_=op[:])
                        row = b * S + ic * IC + it * KI
                        nc.sync.dma_start(
                            part[row:row + KI, nk * 512:(nk + 1) * 512],
                            o_sb[:])

            # b0: K first (scores need all of K), Q(0), then V
            xs0 = []
            for mc in range(NMC):
                x = load_x(0, mc)
                xs0.append(x)
                proj(wk_sb, x, kT_b[0], mc)
            proj(wq_sb, xs0[0], qT_b[0], 0)
            for mc in range(NMC):
                v_proj(xs0[mc], 0, mc)
            xs0 = None

            # window 1: attention(b0) + b1 KV fill + b0 Q JIT
            a0 = []
            for ic in range(NIC):
                a0.append(attn_core(0, ic))
                if ic < NIC - 1:
                    xq = load_x(0, ic + 1)
                    proj(wq_sb, xq, qT_b[0], ic + 1)
                xkv = load_x(1, ic)
                proj(wk_sb, xkv, kT_b[1], ic)
                v_proj(xkv, 1, ic)

            # window 2: attention(b1) + deferred b0 out-proj fill
            xq = load_x(1, 0)
            proj(wq_sb, xq, qT_b[1], 0)
            for ic in range(NIC):
                out_proj(0, ic, a0[ic])
                a1 = attn_core(1, ic)
                if ic < NIC - 1:
                    xq = load_x(1, ic + 1)
                    proj(wq_sb, xq, qT_b[1], ic + 1)
                out_proj(1, ic, a1)

    nc.compile()
    return nc


def _get_nc():
    if "nc" not in _CACHE:
        _CACHE["nc"] = _build_kernel()
    return _CACHE["nc"]


def kernel(x, w_qkv, w_out, b_out):
    import os
    os.environ["BASS_NEVER_TRACE"] = "1"  # NTFF hook absent in this image
    try:
        import jax
        jax.config.update("jax_compilation_cache_dir", "/tmp/jax_nn_attn_cache")
        jax.config.update("jax_persistent_cache_min_compile_time_secs", 1.0)
    except Exception:
        pass
    import ml_dtypes
    from concourse import bass_utils

    x = np.asarray(x, dtype=np.float32)
    w_qkv = np.asarray(w_qkv, dtype=np.float32)
    w_out = np.asarray(w_out, dtype=np.float32)
    b_out = np.asarray(b_out, dtype=np.float32)

    bf16 = ml_dtypes.bfloat16
    xf = np.ascontiguousarray(x.reshape(M, D).T).astype(bf16)
    scale = np.float32(DH ** -0.5)
    in_maps = []
    for c in range(N_CORES):
        cols = slice(c * HC * DH, (c * HC + HC) * DH)
        in_maps.append({
            "xT": xf,
            "wq": (np.ascontiguousarray(w_qkv[:, cols]) * scale).astype(bf16),
            "wk": np.ascontiguousarray(w_qkv[:, D:][:, cols]).astype(bf16),
            "wv": np.ascontiguousarray(w_qkv[:, 2 * D:][:, cols]).astype(bf16),
            "wo": np.ascontiguousarray(w_out[cols, :]).astype(bf16),
        })

    nc = _get_nc()
    res = bass_utils.run_bass_kernel_spmd(
        nc, in_maps, core_ids=list(range(N_CORES)), trace=False)

    total = np.zeros((M, D), np.float32)
    for r in res.results:
        total += r["part"].astype(np.float32)
    total += b_out[None, :]
    return total.reshape(B, S, D)

